# revision 37
# baseline (speedup 1.0000x reference)
"""FEDformer FourierCrossAttention kernel for 8 TRN2 NeuronCores.

Sharding: one head per core (H=8 == n_cores). Each core computes, for its head:
  Q = rfft(q)[:64 modes], K = rfft(k)[:64]      (DFT-as-matmul, hi/lo fp16 3-pass)
  X^T = K^T Q (complex, contract E)             (2-batch 256-col matmuls)
  T = tanh(X) (complex, tau/sin/cos form)       (ACT tanh+sin, DVE cody-waite RR)
  Y = sum_y T[x,y] K[e,y]                       (fp16 matmuls)
  Z = sum_e W[e,o,x] Y[e,x]   (W scaled 2^16)   (dual-accumulate Wr/Wi fp16 matmuls)
  out = irfft(Z / (512*512))  (G scaled 2^24)   (fp16 matmuls; 2^-40 applied on host)

The whole pipeline is split per batch-half (hf): half 0's attn/tanh/output
stages overlap half 1's DMA-paced DFT, and half 0's weight/irfft stages fill
the PE-idle window of half 1's tanh chain.

Batch indexing: global b = 16*hf + 4*g4 + 2*j + par, stored in the attn/tanh
stages at partition half j (pair LSB) and column group cg = 8*hf + 2*g4 + par.
Y/Z/out stages use plain global b ordering.
"""
import numpy as np

import concourse.bass as bass
import concourse.tile as tile
from concourse import bacc, mybir
from concourse.bass_utils import run_bass_kernel_spmd

F32 = mybir.dt.float32
F16 = mybir.dt.float16
F32R = mybir.dt.float32r
F8 = mybir.dt.float8e4
F8E5 = mybir.dt.float8e5
AF = mybir.ActivationFunctionType
OP = mybir.AluOpType

B, L, H, E, O, M = 32, 1024, 8, 64, 64, 64
NCHUNK = 8          # contraction chunks of 128 over L
NHALF = 2           # batch halves of 16 for DFT PSUM
WSHIFT = 16         # W scaled by 2^WSHIFT on host
GSHIFT = 24         # G scaled by 2^GSHIFT on host
OUT_SCALE = 2.0 ** (-WSHIFT - GSHIFT)
HB = B // NHALF     # 16 batches per half

PI = np.float64(np.pi)
PI_HI = np.float32(3.140625)
PI_MID = np.float32(PI - np.float64(np.float32(3.140625)))
PI_LO = np.float32(PI - np.float64(np.float32(3.140625)) - np.float64(PI_MID))
MAGIC = np.float32(1.5 * 2 ** 23)   # round-to-nearest via add/sub
RH_LIM = np.nextafter(np.float32(np.pi) - np.float32(np.pi / 2), np.float32(0))


def build(debug=False):
    nc = bacc.Bacc("TRN2", target_bir_lowering=False, debug=False, num_devices=8)

    # ---- I/O (per-core, host pre-sharded/relaid) ----
    # q/k hi fp16: [c][hf][p][t][col]; t in {qh, kh}, col = b_local*64 + e
    xh_d = nc.dram_tensor("xh", (NCHUNK, NHALF, 128, 2, HB * E), F16,
                          kind="ExternalInput")
    # q/k lo fp8 e4m3, scaled 2^12: t in {ql8, kl8}; the matching DFT matrix
    # is pre-scaled 2^-12 in fp16 (subnormal range is exact enough), so the
    # mixed fp16 x fp8 matmul accumulates into the same PSUM at scale 1.
    xl_d = nc.dram_tensor("xl", (NCHUNK, NHALF, 128, 2, HB * E), F8,
                          kind="ExternalInput")
    # packed fp16 consts: [p][fh(8*128) | fl(8*128) | g(1024) | fh12(8*128)]
    c_d = nc.dram_tensor("cp", (128, 4 * 1024), F16, kind="ExternalInput")
    # W packed fp16 (x2^16): [e][ri][x][o] = W{ri}[e, o, x]
    w_d = nc.dram_tensor("wp", (E, 2, M, O), F16, kind="ExternalInput")
    # transpose helper fp32
    idq_d = nc.dram_tensor("idq", (128, 128), F32, kind="ExternalInput")

    # out[bp][p][l]: p = (pair half)*64 + o; global b = 2*bp + (p>=64)
    out_d = nc.dram_tensor("out", (B // 2, 128, L), F16, kind="ExternalOutput")

    with tile.TileContext(nc) as tc:
        from contextlib import ExitStack
        stack = ExitStack()
        with stack:
            consts = stack.enter_context(tc.tile_pool(name="consts", bufs=1))
            chunks = stack.enter_context(tc.tile_pool(name="chunks", bufs=4))
            coeff = stack.enter_context(tc.tile_pool(name="coeff", bufs=1))
            work = stack.enter_context(tc.tile_pool(name="work", bufs=1))
            tmp = stack.enter_context(tc.tile_pool(name="tmp", bufs=1))
            outs = stack.enter_context(tc.tile_pool(name="outs", bufs=4))
            dft_ps = stack.enter_context(tc.tile_pool(name="dft_ps", bufs=1, space="PSUM"))
            tp_ps = stack.enter_context(tc.tile_pool(name="tp_ps", bufs=2, space="PSUM"))
            at_ps = stack.enter_context(tc.tile_pool(name="at_ps", bufs=2, space="PSUM"))

            # ---------- constants ----------
            c_t = consts.tile([128, 4 * 1024], F16, tag="cp")
            w_t = consts.tile([E, 2, M, O], F16, tag="w")
            idq_t = consts.tile([128, 128], F32, tag="idq")
            # pre-load ACT function set 18 (covers tanh+sin+square+copy):
            # the auto-placer is first-fit (tanh->set0, sin->set9) and would
            # otherwise thrash 1.3us table loads on every tanh<->sin switch.
            _ld = mybir.InstLoadActFuncSet(name=nc.get_next_instruction_name(), ins=[], outs=[])
            _ld.act_func_set_id = 18
            nc.scalar.add_instruction(_ld)
            nc.scalar.dma_start(out=c_t, in_=c_d[:])
            nc.scalar.dma_start(out=w_t, in_=w_d[:])
            nc.scalar.dma_start(out=idq_t, in_=idq_d[:])
            fh_t = c_t[:, 0:1024].rearrange("p (c m) -> p c m", m=2 * M)
            fl_t = c_t[:, 1024:2048].rearrange("p (c m) -> p c m", m=2 * M)
            g_t = c_t[:, 2048:3072]
            f12_t = c_t[:, 3072:4096].rearrange("p (c m) -> p c m", m=2 * M)

            # ---------- persistent state ----------
            qm_h = [coeff.tile([128, 1024], F32, tag=f"qmh{hf}", name=f"qm_h{hf}")
                    for hf in range(NHALF)]
            km_h = [coeff.tile([128, 1024], F32, tag=f"kmh{hf}", name=f"km_h{hf}")
                    for hf in range(NHALF)]
            km16_t = coeff.tile([128, B, E], F16, tag="km16")
            # layout [p=(par,e), ri, bp, y] so stationary attn1 slices merge
            # into a single contiguous free dim (BIR requirement)
            qe_h = [work.tile([128, 2, 8, 64], F32, tag=f"qeh{hf}", name=f"qe_h{hf}") for hf in range(NHALF)]
            ke_h = [work.tile([128, 2, 8, 64], F32, tag=f"keh{hf}", name=f"ke_h{hf}") for hf in range(NHALF)]
            qf_h = [work.tile([128, 2, 8, 64], F32, tag=f"qfh{hf}", name=f"qf_h{hf}") for hf in range(NHALF)]
            # A/B packed: ab[p = 64*j + y, cg, 0:64 = Re X^T, 64:128 = Im X^T]
            ab_t = work.tile([128, 16, 128], F32, tag="ab")
            halfpi = consts.tile([128, 1], F32, tag="halfpi", name="halfpi")
            nc.vector.memset(halfpi[:], float(np.pi / 2))
            t_t = work.tile([128, 16, 128], F16, tag="t")
            tf_t = work.tile([128, 16, 128], F16, tag="tf")
            tt_t = work.tile([128, B, 128], F16, tag="tt")
            tt_v = tt_t[:].rearrange("p (hg j par) c -> p hg j par c", j=2, par=2)
            y_t = work.tile([E, B, 2, M], F16, tag="y")
            yf_t = work.tile([E, B, 2, M], F16, tag="yf")
            z_t = work.tile([O, B, 2, M], F16, tag="z")
            zp_g = [work.tile([128, 8, O], F16, tag=f"zp{g}", name=f"zp_g{g}")
                    for g in range(B // 8)]
            idk16 = consts.tile([64, 64], F16, tag="id16")

            # ---------- stage 5+6, per half ----------
            def tanh_half(hf):
                cgs = slice(8 * hf, 8 * hf + 8)
                # A=Re X^T, B=Im X^T, strided views of ab_t [128, 8, 64]
                av = ab_t[:, cgs, 0:64]
                bv = ab_t[:, cgs, 64:128]
                def ctt(n):
                    return tmp.tile([128, 512], F32, tag="ct", name=f"ct_{n}{hf}", bufs=6)
                def v2(t):
                    return t[:].rearrange("p (g m) -> p g m", m=64)
                ct_n = ctt("n")
                nc.vector.tensor_scalar(v2(ct_n), bv, float(1.0 / PI), float(MAGIC), OP.mult, OP.add)
                nc.vector.tensor_scalar_sub(ct_n[:], ct_n[:], float(MAGIC))
                ct_rh = ctt("rh")
                nc.vector.cody_waite_cascade(v2(ct_rh), bv, ct_n[:], float(PI_HI), float(PI_MID), float(PI_LO))
                # clamp |rh| so rh+pi/2 (cos) and 2*rh (sin) stay in [-pi, pi]
                nc.vector.tensor_scalar(ct_rh[:], ct_rh[:], -float(RH_LIM), float(RH_LIM), OP.max, OP.min)
                # T = tanh(a + ib) = (tau + i*sc*w)/ (tau^2 + cos^2(b)*w)
                # with w = 1 - tau^2; using sc = sin(2rh)/2 and
                # d = tau^2 + 2*cos^2(rh)*(1-tau^2)/2 to skip sin(rh)/s^2.
                ct_tau = ctt("tau")
                nc.scalar.activation(v2(ct_tau), av, AF.Tanh)
                ct_c = ctt("c")
                nc.scalar.activation(ct_c[:], ct_rh[:], AF.Sin, bias=halfpi[:])
                ct_sc2 = ctt("sc2")
                nc.scalar.activation(ct_sc2[:], ct_rh[:], AF.Sin, scale=2.0)
                ct_c2 = ctt("c2")
                nc.scalar.activation(ct_c2[:], ct_c[:], AF.Square)
                ct_t2 = ctt("t2")
                nc.scalar.activation(ct_t2[:], ct_tau[:], AF.Square)
                ct_w2 = ctt("w2")
                nc.vector.tensor_scalar(ct_w2[:], ct_t2[:], -0.5, 0.5, OP.mult, OP.add)
                ct_d = ctt("d")
                nc.vector.tensor_mul(ct_d[:], ct_c2[:], ct_w2[:])
                nc.vector.scalar_tensor_tensor(ct_d[:], ct_d[:], 2.0, ct_t2[:], OP.mult, OP.add)
                ct_r = ctt("r")
                nc.vector.reciprocal(ct_r[:], ct_d[:])
                ct_u = ctt("u")
                nc.vector.tensor_mul(ct_u[:], ct_sc2[:], ct_w2[:])
                # T = [Tr | Ti] fp16 ; Tf = [-Ti | Tr]   (same (j, cg) layout)
                nc.vector.tensor_mul(t_t[:, cgs, 0:64], v2(ct_tau), v2(ct_r))
                nc.vector.tensor_mul(t_t[:, cgs, 64:128], v2(ct_u), v2(ct_r))
                nc.vector.tensor_scalar_mul(tf_t[:, cgs, 0:64], t_t[:, cgs, 64:128], -1.0)
                nc.vector.tensor_copy(tf_t[:, cgs, 64:128], t_t[:, cgs, 0:64])

                # TT assembly for this half (feeds attn2, emitted later):
                # global b = 16hf + 4g4 + 2j + par lives at t[64j:64j+64, cg],
                # cg = 8hf + 2g4 + par; b factors as (hg=(hf,g4), j, par).
                # Parity-matched halves via DVE, others via SWDGE SBUF DMAs.
                hgs = slice(4 * hf, 4 * hf + 4)
                def cg_view(t):
                    return t.rearrange("p (hg par) c -> p hg par c", par=2)
                nc.vector.tensor_copy(tt_v[0:64, hgs, 0, :, :], cg_view(t_t[0:64, cgs, :]))
                nc.vector.tensor_copy(tt_v[64:128, hgs, 1, :, :], cg_view(tf_t[64:128, cgs, :]))
                nc.gpsimd.dma_start(out=tt_v[0:64, hgs, 1, :, :], in_=cg_view(t_t[64:128, cgs, :]))
                nc.gpsimd.dma_start(out=tt_v[64:128, hgs, 0, :, :], in_=cg_view(tf_t[0:64, cgs, :]))

            # ---------- attn2 (PE side), per half ----------
            # emitted in dependency-ready order: PE queues are in-order, so a
            # matmul waiting on the tanh chain must not be emitted before PE
            # work whose inputs are already available.
            def attn2_half(hf, copy_eng):
                # Y in plain global-b order
                for b4 in range(4 * hf, 4 * hf + 4):
                    yp = at_ps.tile([E, 4, 128], F32, tag="pt", bufs=2, name=f"yp{b4}")
                    for j in range(4):
                        b = b4 * 4 + j
                        nc.tensor.matmul(yp[:, j, :], km16_t[:, b, :], tt_t[:, b, :],
                                         start=True, stop=True)
                    dst = y_t[:, b4 * 4:(b4 + 1) * 4, :, :]
                    srcv = yp[:].rearrange("p b (ri m) -> p b ri m", m=M)
                    if copy_eng == "dve" or (copy_eng == "mix" and b4 % 2 == 1):
                        nc.vector.tensor_copy(dst, srcv)
                    else:
                        nc.scalar.copy(dst, srcv)
                # Yf = [-Yi | Yr] for the dual-accumulate weight stage
                hb = slice(16 * hf, 16 * hf + 16)
                if copy_eng == "act":
                    nc.scalar.mul(yf_t[:, hb, 0, :], y_t[:, hb, 1, :], -1.0)
                    nc.scalar.copy(yf_t[:, hb, 1, :], y_t[:, hb, 0, :])
                elif copy_eng == "dve":
                    nc.vector.tensor_scalar_mul(yf_t[:, hb, 0, :], y_t[:, hb, 1, :], -1.0)
                    nc.vector.tensor_copy(yf_t[:, hb, 1, :], y_t[:, hb, 0, :])
                else:
                    nc.vector.tensor_scalar_mul(yf_t[:, hb, 0, :], y_t[:, hb, 1, :], -1.0)
                    nc.scalar.copy(yf_t[:, hb, 1, :], y_t[:, hb, 0, :])

            # ---------- stages 7-9, per half ----------
            # Zr = Wr^T Yr - Wi^T Yi ; Zi = Wr^T Yi + Wi^T Yr, via two
            # accumulating matmuls: Wr^T @ [Yr|Yi] + Wi^T @ [-Yi|Yr].
            # Then Z transposes -> Z' [(ri,x), (b, o)] and irfft out = Z'^T G.
            # PSUM comes from the transpose tag (free once transposes done).
            def stage789_half(hf, z_eng, out_eng):
                b0 = 16 * hf
                for x8 in range(M // 8):
                    wp = tp_ps.tile([O, 8, HB * 2], F32, tag="tp", bufs=2,
                                    name=f"wp{hf}_{x8}")
                    for j in range(8):
                        x = x8 * 8 + j
                        yv = y_t[:, b0:b0 + HB, :, x].rearrange("p b ri -> p (b ri)")
                        yfv = yf_t[:, b0:b0 + HB, :, x].rearrange("p b ri -> p (b ri)")
                        nc.tensor.matmul(wp[:, j, :], w_t[:, 0, x, :], yv,
                                         start=True, stop=False)
                        nc.tensor.matmul(wp[:, j, :], w_t[:, 1, x, :], yfv,
                                         start=False, stop=True)
                    dst = z_t[:, b0:b0 + HB, :, x8 * 8:(x8 + 1) * 8].rearrange("p b ri x -> p x b ri")
                    srcv = wp[:].rearrange("p x (b ri) -> p x b ri", ri=2)
                    if z_eng == "act" or x8 % 2 == 0:
                        nc.scalar.copy(dst, srcv)
                    else:
                        nc.vector.tensor_copy(dst, srcv)

                for b8 in range(2 * hf, 2 * hf + 2):
                    zt = tp_ps.tile([128, 8, O], F16, tag="tp", bufs=2,
                                    name=f"zt{b8}")
                    for j in range(8):
                        b = b8 * 8 + j
                        nc.tensor.transpose(
                            zt[:, j, :],
                            z_t[:, b, :, :].rearrange("p ri m -> p (ri m)"),
                            idk16[:],
                        )
                    if z_eng == "act" or b8 % 2 == 1:
                        nc.scalar.copy(zp_g[b8][:], zt[:])
                    else:
                        nc.vector.tensor_copy(zp_g[b8][:], zt[:])

                # irfft + staged fp16 output (host applies OUT_SCALE;
                # fp16 can't hold out*2^-40 without underflow)
                for bp in range(8 * hf, 8 * hf + 8):
                    otg = outs.tile([128, 1024], F16, tag="ot", name=f"ot{bp}")
                    for gg in range(2):
                        opg = dft_ps.tile([128, 512], F32,
                                          tag=("qmps" if (bp + gg) % 2 == 0 else "kmps"),
                                          bufs=1, name=f"op{bp}_{gg}")
                        nc.tensor.matmul(
                            opg[:, :],
                            zp_g[bp // 4][:, (bp % 4) * 2:(bp % 4) * 2 + 2, :]
                            .rearrange("p b o -> p (b o)"),
                            g_t[:, gg * 512:(gg + 1) * 512],
                            start=True, stop=True,
                        )
                        if out_eng == "act" or (bp + gg) % 2 == 0:
                            nc.scalar.copy(otg[:, gg * 512:(gg + 1) * 512], opg[:])
                        else:
                            nc.vector.tensor_copy(otg[:, gg * 512:(gg + 1) * 512], opg[:])
                        nc.sync.dma_start(out=out_d[bp][:, gg * 512:(gg + 1) * 512],
                                          in_=otg[:, gg * 512:(gg + 1) * 512])

            # ---------- main per-half pipeline ----------
            def dft_half(hf):
                # ----- stage 1+2: DFT (hi/lo 3-pass) -----
                qm_ps = dft_ps.tile([128, 1024], F32, tag="qmps", name=f"qm_ps{hf}", bufs=1)
                km_ps = dft_ps.tile([128, 1024], F32, tag="kmps", name=f"km_ps{hf}", bufs=1)
                for c in range(NCHUNK):
                    xh_c = chunks.tile([128, 2, HB * E], F16, tag="xh", name=f"xh{hf}_{c}")
                    xl_c = chunks.tile([128, 2, HB * E], F8, tag="xl", name=f"xl{hf}_{c}")
                    nc.sync.dma_start(out=xh_c, in_=xh_d[c, hf])
                    nc.sync.dma_start(out=xl_c, in_=xl_d[c, hf])
                    first = c == 0
                    last = c == NCHUNK - 1
                    passes = (
                        (fh_t[:, c, :], xh_c, 0, qm_ps, first, False),
                        (fh_t[:, c, :], xh_c, 1, km_ps, first, False),
                        (fl_t[:, c, :], xh_c, 0, qm_ps, False, False),
                        (fl_t[:, c, :], xh_c, 1, km_ps, False, False),
                        (f12_t[:, c, :], xl_c, 0, qm_ps, False, last),
                        (f12_t[:, c, :], xl_c, 1, km_ps, False, last),
                    )
                    for lhs, src, ti, ps, is_start, is_stop in passes:
                        for g in range(2):
                            nc.tensor.matmul(
                                ps[:, g * 512:(g + 1) * 512],
                                lhs,
                                src[:, ti, g * 512:(g + 1) * 512],
                                start=is_start,
                                stop=is_stop,
                            )
                nc.vector.tensor_copy(qm_h[hf][:], qm_ps[:])
                nc.scalar.copy(km_h[hf][:], km_ps[:])
                nc.vector.tensor_copy(
                    km16_t[:, hf * HB:(hf + 1) * HB, :],
                    km_ps[:].rearrange("p (b e) -> p b e", e=E),
                )
                if hf == 0:
                    nc.vector.tensor_copy(idk16[:], idq_t[0:64, 0:64])

            def tr_group(g):
                hf = g // 2
                # ----- stage 3: pair transposes -> Q_e, K_e -----
                # in [2m, (b0-e|b1-e)] -> out [(b0-e|b1-e), 2m]; even b on
                # partitions 0:64, odd on 64:128.
                qm_p = qm_h[hf][:].rearrange("p (bp c) -> p bp c", c=128)
                km_p = km_h[hf][:].rearrange("p (bp c) -> p bp c", c=128)
                for g2 in range(2 * (g % 2), 2 * (g % 2) + 2):
                    tp = tp_ps.tile([128, 2, 128], F32, tag="tp", name=f"tp{g}_{g2}")
                    tk = tp_ps.tile([128, 2, 128], F32, tag="tp", name=f"tk{g}_{g2}")
                    for j in range(2):
                        bpl = g2 * 2 + j
                        nc.tensor.transpose(tp[:, j, :], qm_p[:, bpl, :], idq_t[:])
                        nc.tensor.transpose(tk[:, j, :], km_p[:, bpl, :], idq_t[:])
                    tpv = tp[:].rearrange("p j (ri y) -> p ri j y", ri=2)
                    tkv = tk[:].rearrange("p j (ri y) -> p ri j y", ri=2)
                    if g2 % 2 == 0:
                        nc.scalar.copy(qe_h[hf][:, :, g2 * 2:(g2 + 1) * 2, :], tpv)
                        nc.scalar.copy(ke_h[hf][:, :, g2 * 2:(g2 + 1) * 2, :], tkv)
                    else:
                        nc.vector.tensor_copy(qe_h[hf][:, :, g2 * 2:(g2 + 1) * 2, :], tpv)
                        nc.vector.tensor_copy(ke_h[hf][:, :, g2 * 2:(g2 + 1) * 2, :], tkv)
                bsl = slice(4 * (g % 2), 4 * (g % 2) + 4)
                nc.vector.tensor_scalar_mul(qf_h[hf][:, 0, bsl, :], qe_h[hf][:, 1, bsl, :], -1.0)
                nc.vector.tensor_copy(qf_h[hf][:, 1, bsl, :], qe_h[hf][:, 0, bsl, :])

            def attn1_group(g):
                hf = g // 2
                # ----- stage 4: attn1 -> X^T psum, A/B fp32 sbuf -----
                # 2 same-parity b per matmul pair (256 cols each); useful
                # quadrants j == j'; partition-aligned extraction.
                for par in range(2):
                    base = 64 * par
                    sl = slice(base, base + 64)
                    for g4 in range(2 * (g % 2), 2 * (g % 2) + 2):
                        pt = at_ps.tile([128, 2, 2, 64], F32, tag="pt", bufs=2,
                                        name=f"pt{g}_{par}_{g4}")
                        psl = slice(2 * g4, 2 * g4 + 2)
                        nc.tensor.matmul(pt[:], ke_h[hf][sl, 0, psl, :],
                                         qe_h[hf][sl, :, psl, :],
                                         start=True, stop=False)
                        nc.tensor.matmul(pt[:], ke_h[hf][sl, 1, psl, :],
                                         qf_h[hf][sl, :, psl, :],
                                         start=False, stop=True)
                        cg = 8 * hf + 2 * g4 + par
                        if (par + g4) % 2 == 0:
                            nc.scalar.copy(ab_t[0:64, cg, :].rearrange("p (ri y) -> p ri y", ri=2), pt[0:64, :, 0, :])
                            nc.vector.tensor_copy(ab_t[64:128, cg, :].rearrange("p (ri y) -> p ri y", ri=2), pt[64:128, :, 1, :])
                        else:
                            nc.vector.tensor_copy(ab_t[0:64, cg, :].rearrange("p (ri y) -> p ri y", ri=2), pt[0:64, :, 0, :])
                            nc.scalar.copy(ab_t[64:128, cg, :].rearrange("p (ri y) -> p ri y", ri=2), pt[64:128, :, 1, :])

            # Emission order = PE dependency-ready order (PE queues are
            # in-order; a stalled head blocks everything behind it).
            dft_half(0)
            for g in (0, 1):
                tr_group(g)
                attn1_group(g)
            tanh_half(0)
            dft_half(1)
            attn2_half(0, "act")
            for g in (2, 3):
                tr_group(g)
                attn1_group(g)
            tanh_half(1)
            stage789_half(0, "act", "mix")
            attn2_half(1, "mix")
            stage789_half(1, "mix", "mix")

    nc.compile()
    return nc


_NC_CACHE = None


def _get_nc():
    global _NC_CACHE
    if _NC_CACHE is None:
        _NC_CACHE = build()
    return _NC_CACHE


def _host_prep(q, k, Wr, Wi):
    """Build the 8 per-core input maps (numpy relayout/cast only)."""
    l = np.arange(L, dtype=np.float64)[:, None]
    m = np.arange(M, dtype=np.float64)[None, :]
    ang = 2.0 * np.pi * l * m / L
    F = np.concatenate([np.cos(ang), -np.sin(ang)], axis=1).astype(np.float32)  # [L, 2M]
    fh = F.astype(np.float16)
    fl = (F - fh.astype(np.float32)).astype(np.float16)
    # fh/fl as [p][(c, 2m)]
    fh = fh.reshape(NCHUNK, 128, 2 * M).transpose(1, 0, 2).reshape(128, 1024)
    fl = fl.reshape(NCHUNK, 128, 2 * M).transpose(1, 0, 2).reshape(128, 1024)

    cm = np.full(M, 2.0); cm[0] = 1.0
    ang2 = 2.0 * np.pi * m.T * np.arange(L, dtype=np.float64)[None, :] / L
    SC = 2.0 ** GSHIFT / (L * 512.0 * 512.0)
    g = np.concatenate([
        cm[:, None] * np.cos(ang2) * SC,
        -cm[:, None] * np.sin(ang2) * SC,
    ], axis=0).astype(np.float32).astype(np.float16)  # [2M, L]

    f12 = (F * 2.0 ** -12).astype(np.float16)
    f12 = f12.reshape(NCHUNK, 128, 2 * M).transpose(1, 0, 2).reshape(128, 1024)
    cpack = np.concatenate([fh, fl, g.astype(np.float16), f12], axis=1)  # [128, 4096]

    idq = np.eye(128, dtype=np.float32)

    from ml_dtypes import float8_e4m3fn as E4M3
    maps = []
    for h in range(H):
        def split(x):
            xs = np.ascontiguousarray(x[:, :, h, :].transpose(1, 0, 2)).reshape(L, B * E)
            hi = xs.astype(np.float16)
            lo = ((xs - hi.astype(np.float32)) * 2.0 ** 12).astype(E4M3)
            return hi, lo
        qh, ql8 = split(q)
        kh, kl8 = split(k)
        # pack [c][hf][p][t][col]
        xph = np.empty((NCHUNK, NHALF, 128, 2, HB * E), np.float16)
        xpl = np.empty((NCHUNK, NHALF, 128, 2, HB * E), E4M3)
        for t, (dst, src) in enumerate((( xph, qh), (xph, kh), (xpl, ql8), (xpl, kl8))):
            sv = src.reshape(NCHUNK, 128, NHALF, HB * E)
            dst[:, :, :, t % 2, :] = sv.transpose(0, 2, 1, 3)
        wpk = np.empty((E, 2, M, O), np.float32)
        wpk[:, 0] = (Wr[h] * 2.0 ** WSHIFT).transpose(0, 2, 1)  # [e,o,x]->[e,x,o]
        wpk[:, 1] = (Wi[h] * 2.0 ** WSHIFT).transpose(0, 2, 1)
        maps.append({
            "xh": xph,
            "xl": xpl,
            "cp": cpack,
            "wp": wpk.astype(np.float16),
            "idq": idq,
        })
    return maps


def kernel(q, k, v, Wr, Wi, _trace=False):
    q = np.asarray(q, np.float32)
    k = np.asarray(k, np.float32)
    Wr = np.asarray(Wr, np.float32)
    Wi = np.asarray(Wi, np.float32)
    nc = _get_nc()
    maps = _host_prep(q, k, Wr, Wi)
    try:
        res = run_bass_kernel_spmd(nc, maps, core_ids=list(range(H)), trace=_trace)
    except ModuleNotFoundError:
        res = run_bass_kernel_spmd(nc, maps, core_ids=list(range(H)), trace=False)
    # out_d[bp][p][l]: b = 2*bp + (p//64), o = p%64 -> plain b order
    out = np.empty((B, H, O, L), np.float32)
    for h in range(H):
        o = np.asarray(res.results[h]["out"], np.float32).reshape(B, O, L)
        o *= np.float32(OUT_SCALE)
        out[:, h] = o
    if _trace:
        kernel.last_results = res
    return out.astype(np.float32)


# revision 38
# speedup vs baseline: 1.0129x; 1.0129x over previous
"""FEDformer FourierCrossAttention kernel for 8 TRN2 NeuronCores.

Sharding: one head per core (H=8 == n_cores). Each core computes, for its head:
  Q = rfft(q)[:64 modes], K = rfft(k)[:64]      (DFT-as-matmul, hi/lo fp16 3-pass)
  X^T = K^T Q (complex, contract E)             (2-batch 256-col matmuls)
  T = tanh(X) (complex, tau/sin/cos form)       (ACT tanh+sin, DVE cody-waite RR)
  Y = sum_y T[x,y] K[e,y]                       (fp16 matmuls)
  Z = sum_e W[e,o,x] Y[e,x]   (W scaled 2^16)   (dual-accumulate Wr/Wi fp16 matmuls)
  out = irfft(Z / (512*512))  (G scaled 2^24)   (fp16 matmuls; 2^-40 applied on host)

The whole pipeline is split per batch-half (hf): half 0's attn/tanh/output
stages overlap half 1's DMA-paced DFT, and half 0's weight/irfft stages fill
the PE-idle window of half 1's tanh chain.

Batch indexing: global b = 16*hf + 4*g4 + 2*j + par, stored in the attn/tanh
stages at partition half j (pair LSB) and column group cg = 8*hf + 2*g4 + par.
Y/Z/out stages use plain global b ordering.
"""
import numpy as np

import concourse.bass as bass
import concourse.tile as tile
from concourse import bacc, mybir
from concourse.bass_utils import run_bass_kernel_spmd

F32 = mybir.dt.float32
F16 = mybir.dt.float16
F32R = mybir.dt.float32r
F8 = mybir.dt.float8e4
F8E5 = mybir.dt.float8e5
AF = mybir.ActivationFunctionType
OP = mybir.AluOpType

B, L, H, E, O, M = 32, 1024, 8, 64, 64, 64
NCHUNK = 8          # contraction chunks of 128 over L
NHALF = 2           # batch halves of 16 for DFT PSUM
WSHIFT = 16         # W scaled by 2^WSHIFT on host
GSHIFT = 24         # G scaled by 2^GSHIFT on host
OUT_SCALE = 2.0 ** (-WSHIFT - GSHIFT)
HB = B // NHALF     # 16 batches per half

PI = np.float64(np.pi)
PI_HI = np.float32(3.140625)
PI_MID = np.float32(PI - np.float64(np.float32(3.140625)))
PI_LO = np.float32(PI - np.float64(np.float32(3.140625)) - np.float64(PI_MID))
MAGIC = np.float32(1.5 * 2 ** 23)   # round-to-nearest via add/sub
RH_LIM = np.nextafter(np.float32(np.pi) - np.float32(np.pi / 2), np.float32(0))


def build(debug=False):
    nc = bacc.Bacc("TRN2", target_bir_lowering=False, debug=False, num_devices=8)

    # ---- I/O (per-core, host pre-sharded/relaid) ----
    # q/k hi fp16: [c][hf][p][t][col]; t in {qh, kh}, col = b_local*64 + e
    xh_d = nc.dram_tensor("xh", (NCHUNK, NHALF, 128, 2, HB * E), F16,
                          kind="ExternalInput")
    # q/k lo fp8 e4m3, scaled 2^12: t in {ql8, kl8}; the matching DFT matrix
    # is pre-scaled 2^-12 in fp16 (subnormal range is exact enough), so the
    # mixed fp16 x fp8 matmul accumulates into the same PSUM at scale 1.
    xl_d = nc.dram_tensor("xl", (NCHUNK, NHALF, 128, 2, HB * E), F8,
                          kind="ExternalInput")
    # packed fp16 consts: [p][fh(8*128) | fl(8*128) | g(1024) | fh12(8*128)]
    c_d = nc.dram_tensor("cp", (128, 4 * 1024), F16, kind="ExternalInput")
    # W packed fp16 (x2^16): [e][ri][x][o] = W{ri}[e, o, x]
    w_d = nc.dram_tensor("wp", (E, 2, M, O), F16, kind="ExternalInput")
    # transpose helper fp32
    idq_d = nc.dram_tensor("idq", (128, 128), F32, kind="ExternalInput")

    # out[bp][p][l]: p = (pair half)*64 + o; global b = 2*bp + (p>=64)
    out_d = nc.dram_tensor("out", (B // 2, 128, L), F16, kind="ExternalOutput")

    with tile.TileContext(nc) as tc:
        from contextlib import ExitStack
        stack = ExitStack()
        with stack:
            consts = stack.enter_context(tc.tile_pool(name="consts", bufs=1))
            chunks = stack.enter_context(tc.tile_pool(name="chunks", bufs=4))
            coeff = stack.enter_context(tc.tile_pool(name="coeff", bufs=1))
            work = stack.enter_context(tc.tile_pool(name="work", bufs=1))
            tmp = stack.enter_context(tc.tile_pool(name="tmp", bufs=1))
            outs = stack.enter_context(tc.tile_pool(name="outs", bufs=4))
            dft_ps = stack.enter_context(tc.tile_pool(name="dft_ps", bufs=1, space="PSUM"))
            tp_ps = stack.enter_context(tc.tile_pool(name="tp_ps", bufs=2, space="PSUM"))
            at_ps = stack.enter_context(tc.tile_pool(name="at_ps", bufs=2, space="PSUM"))

            # ---------- constants ----------
            c_t = consts.tile([128, 4 * 1024], F16, tag="cp")
            w_t = consts.tile([E, 2, M, O], F16, tag="w")
            idq_t = consts.tile([128, 128], F32, tag="idq")
            # pre-load ACT function set 18 (covers tanh+sin+square+copy):
            # the auto-placer is first-fit (tanh->set0, sin->set9) and would
            # otherwise thrash 1.3us table loads on every tanh<->sin switch.
            _ld = mybir.InstLoadActFuncSet(name=nc.get_next_instruction_name(), ins=[], outs=[])
            _ld.act_func_set_id = 18
            nc.scalar.add_instruction(_ld)
            nc.scalar.dma_start(out=c_t, in_=c_d[:])
            nc.scalar.dma_start(out=w_t, in_=w_d[:])
            nc.scalar.dma_start(out=idq_t, in_=idq_d[:])
            fh_t = c_t[:, 0:1024].rearrange("p (c m) -> p c m", m=2 * M)
            fl_t = c_t[:, 1024:2048].rearrange("p (c m) -> p c m", m=2 * M)
            g_t = c_t[:, 2048:3072]
            f12_t = c_t[:, 3072:4096].rearrange("p (c m) -> p c m", m=2 * M)

            # ---------- persistent state ----------
            qm_h = [coeff.tile([128, 1024], F32, tag=f"qmh{hf}", name=f"qm_h{hf}")
                    for hf in range(NHALF)]
            km_h = [coeff.tile([128, 1024], F32, tag=f"kmh{hf}", name=f"km_h{hf}")
                    for hf in range(NHALF)]
            km16_t = coeff.tile([128, B, E], F16, tag="km16")
            # layout [p=(par,e), ri, bp, y] so stationary attn1 slices merge
            # into a single contiguous free dim (BIR requirement)
            qe_h = [work.tile([128, 2, 8, 64], F32, tag=f"qeh{hf}", name=f"qe_h{hf}") for hf in range(NHALF)]
            ke_h = [work.tile([128, 2, 8, 64], F32, tag=f"keh{hf}", name=f"ke_h{hf}") for hf in range(NHALF)]
            qf_h = [work.tile([128, 2, 8, 64], F32, tag=f"qfh{hf}", name=f"qf_h{hf}") for hf in range(NHALF)]
            # A/B packed: ab[p = 64*j + y, cg, 0:64 = Re X^T, 64:128 = Im X^T]
            ab_t = work.tile([128, 16, 128], F32, tag="ab")
            halfpi = consts.tile([128, 1], F32, tag="halfpi", name="halfpi")
            nc.vector.memset(halfpi[:], float(np.pi / 2))
            t_t = work.tile([128, 16, 128], F16, tag="t")
            tf_t = work.tile([128, 16, 128], F16, tag="tf")
            tt_t = work.tile([128, B, 128], F16, tag="tt")
            tt_v = tt_t[:].rearrange("p (hg j par) c -> p hg j par c", j=2, par=2)
            y_t = work.tile([E, B, 2, M], F16, tag="y")
            yf_t = work.tile([E, B, 2, M], F16, tag="yf")
            z_t = work.tile([O, B, 2, M], F16, tag="z")
            zp_g = [work.tile([128, 8, O], F16, tag=f"zp{g}", name=f"zp_g{g}")
                    for g in range(B // 8)]
            idk16 = consts.tile([64, 64], F16, tag="id16")

            # ---------- stage 5+6, per half ----------
            def tanh_half(hf):
                cgs = slice(8 * hf, 8 * hf + 8)
                # A=Re X^T, B=Im X^T, strided views of ab_t [128, 8, 64]
                av = ab_t[:, cgs, 0:64]
                bv = ab_t[:, cgs, 64:128]
                def ctt(n):
                    return tmp.tile([128, 512], F32, tag="ct", name=f"ct_{n}{hf}", bufs=6)
                def v2(t):
                    return t[:].rearrange("p (g m) -> p g m", m=64)
                ct_n = ctt("n")
                nc.vector.tensor_scalar(v2(ct_n), bv, float(1.0 / PI), float(MAGIC), OP.mult, OP.add)
                nc.vector.tensor_scalar_sub(ct_n[:], ct_n[:], float(MAGIC))
                ct_rh = ctt("rh")
                nc.vector.cody_waite_cascade(v2(ct_rh), bv, ct_n[:], float(PI_HI), float(PI_MID), float(PI_LO))
                # clamp |rh| so rh+pi/2 (cos) and 2*rh (sin) stay in [-pi, pi]
                nc.vector.tensor_scalar(ct_rh[:], ct_rh[:], -float(RH_LIM), float(RH_LIM), OP.max, OP.min)
                # T = tanh(a + ib) = (tau + i*sc*w)/ (tau^2 + cos^2(b)*w)
                # with w = 1 - tau^2; using sc = sin(2rh)/2 and
                # d = tau^2 + 2*cos^2(rh)*(1-tau^2)/2 to skip sin(rh)/s^2.
                ct_tau = ctt("tau")
                nc.scalar.activation(v2(ct_tau), av, AF.Tanh)
                ct_c = ctt("c")
                nc.scalar.activation(ct_c[:], ct_rh[:], AF.Sin, bias=halfpi[:])
                ct_sc2 = ctt("sc2")
                nc.scalar.activation(ct_sc2[:], ct_rh[:], AF.Sin, scale=2.0)
                ct_c2 = ctt("c2")
                nc.scalar.activation(ct_c2[:], ct_c[:], AF.Square)
                ct_t2 = ctt("t2")
                nc.scalar.activation(ct_t2[:], ct_tau[:], AF.Square)
                ct_w2 = ctt("w2")
                nc.vector.tensor_scalar(ct_w2[:], ct_t2[:], -0.5, 0.5, OP.mult, OP.add)
                ct_d = ctt("d")
                nc.vector.tensor_mul(ct_d[:], ct_c2[:], ct_w2[:])
                nc.vector.scalar_tensor_tensor(ct_d[:], ct_d[:], 2.0, ct_t2[:], OP.mult, OP.add)
                ct_r = ctt("r")
                nc.vector.reciprocal(ct_r[:], ct_d[:])
                ct_u = ctt("u")
                nc.vector.tensor_mul(ct_u[:], ct_sc2[:], ct_w2[:])
                # T = [Tr | Ti] fp16 ; Tf = [-Ti | Tr]   (same (j, cg) layout)
                nc.vector.tensor_mul(t_t[:, cgs, 0:64], v2(ct_tau), v2(ct_r))
                nc.vector.tensor_mul(t_t[:, cgs, 64:128], v2(ct_u), v2(ct_r))
                nc.vector.tensor_scalar_mul(tf_t[:, cgs, 0:64], t_t[:, cgs, 64:128], -1.0)
                nc.vector.tensor_copy(tf_t[:, cgs, 64:128], t_t[:, cgs, 0:64])

                # TT assembly for this half (feeds attn2, emitted later):
                # global b = 16hf + 4g4 + 2j + par lives at t[64j:64j+64, cg],
                # cg = 8hf + 2g4 + par; b factors as (hg=(hf,g4), j, par).
                # Parity-matched halves via DVE, others via SWDGE SBUF DMAs.
                hgs = slice(4 * hf, 4 * hf + 4)
                def cg_view(t):
                    return t.rearrange("p (hg par) c -> p hg par c", par=2)
                nc.vector.tensor_copy(tt_v[0:64, hgs, 0, :, :], cg_view(t_t[0:64, cgs, :]))
                nc.vector.tensor_copy(tt_v[64:128, hgs, 1, :, :], cg_view(tf_t[64:128, cgs, :]))
                nc.gpsimd.dma_start(out=tt_v[0:64, hgs, 1, :, :], in_=cg_view(t_t[64:128, cgs, :]))
                nc.gpsimd.dma_start(out=tt_v[64:128, hgs, 0, :, :], in_=cg_view(tf_t[0:64, cgs, :]))

            # ---------- attn2 (PE side), per half ----------
            # emitted in dependency-ready order: PE queues are in-order, so a
            # matmul waiting on the tanh chain must not be emitted before PE
            # work whose inputs are already available.
            def attn2_half(hf, copy_eng):
                # Y in plain global-b order
                for b4 in range(4 * hf, 4 * hf + 4):
                    yp = at_ps.tile([E, 4, 128], F32, tag="pt", bufs=2, name=f"yp{b4}")
                    for j in range(4):
                        b = b4 * 4 + j
                        nc.tensor.matmul(yp[:, j, :], km16_t[:, b, :], tt_t[:, b, :],
                                         start=True, stop=True)
                    dst = y_t[:, b4 * 4:(b4 + 1) * 4, :, :]
                    srcv = yp[:].rearrange("p b (ri m) -> p b ri m", m=M)
                    if copy_eng == "dve" or (copy_eng == "mix" and b4 % 2 == 1):
                        nc.vector.tensor_copy(dst, srcv)
                    else:
                        nc.scalar.copy(dst, srcv)
                # Yf = [-Yi | Yr] for the dual-accumulate weight stage
                hb = slice(16 * hf, 16 * hf + 16)
                if copy_eng == "act":
                    nc.scalar.mul(yf_t[:, hb, 0, :], y_t[:, hb, 1, :], -1.0)
                    nc.scalar.copy(yf_t[:, hb, 1, :], y_t[:, hb, 0, :])
                elif copy_eng == "dve":
                    nc.vector.tensor_scalar_mul(yf_t[:, hb, 0, :], y_t[:, hb, 1, :], -1.0)
                    nc.vector.tensor_copy(yf_t[:, hb, 1, :], y_t[:, hb, 0, :])
                else:
                    nc.vector.tensor_scalar_mul(yf_t[:, hb, 0, :], y_t[:, hb, 1, :], -1.0)
                    nc.scalar.copy(yf_t[:, hb, 1, :], y_t[:, hb, 0, :])

            # ---------- stages 7-9, per half ----------
            # Zr = Wr^T Yr - Wi^T Yi ; Zi = Wr^T Yi + Wi^T Yr, via two
            # accumulating matmuls: Wr^T @ [Yr|Yi] + Wi^T @ [-Yi|Yr].
            # Then Z transposes -> Z' [(ri,x), (b, o)] and irfft out = Z'^T G.
            # PSUM comes from the transpose tag (free once transposes done).
            def stage789_half(hf, z_eng, out_eng):
                b0 = 16 * hf
                for x8 in range(M // 8):
                    wp = tp_ps.tile([O, 8, HB * 2], F32, tag="tp", bufs=2,
                                    name=f"wp{hf}_{x8}")
                    for j in range(8):
                        x = x8 * 8 + j
                        yv = y_t[:, b0:b0 + HB, :, x].rearrange("p b ri -> p (b ri)")
                        yfv = yf_t[:, b0:b0 + HB, :, x].rearrange("p b ri -> p (b ri)")
                        nc.tensor.matmul(wp[:, j, :], w_t[:, 0, x, :], yv,
                                         start=True, stop=False)
                        nc.tensor.matmul(wp[:, j, :], w_t[:, 1, x, :], yfv,
                                         start=False, stop=True)
                    dst = z_t[:, b0:b0 + HB, :, x8 * 8:(x8 + 1) * 8].rearrange("p b ri x -> p x b ri")
                    srcv = wp[:].rearrange("p x (b ri) -> p x b ri", ri=2)
                    if z_eng == "act" or x8 % 2 == 0:
                        nc.scalar.copy(dst, srcv)
                    else:
                        nc.vector.tensor_copy(dst, srcv)

                for b8 in range(2 * hf, 2 * hf + 2):
                    zt = tp_ps.tile([128, 8, O], F16, tag="tp", bufs=2,
                                    name=f"zt{b8}")
                    for j in range(8):
                        b = b8 * 8 + j
                        nc.tensor.transpose(
                            zt[:, j, :],
                            z_t[:, b, :, :].rearrange("p ri m -> p (ri m)"),
                            idk16[:],
                        )
                    if z_eng == "act" or b8 % 2 == 1:
                        nc.scalar.copy(zp_g[b8][:], zt[:])
                    else:
                        nc.vector.tensor_copy(zp_g[b8][:], zt[:])

                # irfft + staged fp16 output (host applies OUT_SCALE;
                # fp16 can't hold out*2^-40 without underflow)
                for bp in range(8 * hf, 8 * hf + 8):
                    otg = outs.tile([128, 1024], F16, tag="ot", name=f"ot{bp}")
                    for gg in range(2):
                        opg = dft_ps.tile([128, 512], F32,
                                          tag=("qmps" if (bp + gg) % 2 == 0 else "kmps"),
                                          bufs=1, name=f"op{bp}_{gg}")
                        nc.tensor.matmul(
                            opg[:, :],
                            zp_g[bp // 4][:, (bp % 4) * 2:(bp % 4) * 2 + 2, :]
                            .rearrange("p b o -> p (b o)"),
                            g_t[:, gg * 512:(gg + 1) * 512],
                            start=True, stop=True,
                        )
                        if out_eng == "act" or (bp + gg) % 2 == 0:
                            nc.scalar.copy(otg[:, gg * 512:(gg + 1) * 512], opg[:])
                        else:
                            nc.vector.tensor_copy(otg[:, gg * 512:(gg + 1) * 512], opg[:])
                    nc.sync.dma_start(out=out_d[bp], in_=otg[:])

            # ---------- main per-half pipeline ----------
            def dft_half(hf):
                # ----- stage 1+2: DFT (hi/lo 3-pass) -----
                qm_ps = dft_ps.tile([128, 1024], F32, tag="qmps", name=f"qm_ps{hf}", bufs=1)
                km_ps = dft_ps.tile([128, 1024], F32, tag="kmps", name=f"km_ps{hf}", bufs=1)
                for c in range(NCHUNK):
                    xh_c = chunks.tile([128, 2, HB * E], F16, tag="xh", name=f"xh{hf}_{c}")
                    xl_c = chunks.tile([128, 2, HB * E], F8, tag="xl", name=f"xl{hf}_{c}")
                    nc.sync.dma_start(out=xh_c, in_=xh_d[c, hf])
                    nc.sync.dma_start(out=xl_c, in_=xl_d[c, hf])
                    first = c == 0
                    last = c == NCHUNK - 1
                    passes = (
                        (fh_t[:, c, :], xh_c, 0, qm_ps, first, False),
                        (fh_t[:, c, :], xh_c, 1, km_ps, first, False),
                        (fl_t[:, c, :], xh_c, 0, qm_ps, False, False),
                        (fl_t[:, c, :], xh_c, 1, km_ps, False, False),
                        (f12_t[:, c, :], xl_c, 0, qm_ps, False, last),
                        (f12_t[:, c, :], xl_c, 1, km_ps, False, last),
                    )
                    for lhs, src, ti, ps, is_start, is_stop in passes:
                        for g in range(2):
                            nc.tensor.matmul(
                                ps[:, g * 512:(g + 1) * 512],
                                lhs,
                                src[:, ti, g * 512:(g + 1) * 512],
                                start=is_start,
                                stop=is_stop,
                            )
                nc.vector.tensor_copy(qm_h[hf][:], qm_ps[:])
                nc.scalar.copy(km_h[hf][:], km_ps[:])
                nc.vector.tensor_copy(
                    km16_t[:, hf * HB:(hf + 1) * HB, :],
                    km_ps[:].rearrange("p (b e) -> p b e", e=E),
                )
                if hf == 0:
                    nc.vector.tensor_copy(idk16[:], idq_t[0:64, 0:64])

            def tr_group(g):
                hf = g // 2
                # ----- stage 3: pair transposes -> Q_e, K_e -----
                # in [2m, (b0-e|b1-e)] -> out [(b0-e|b1-e), 2m]; even b on
                # partitions 0:64, odd on 64:128.
                qm_p = qm_h[hf][:].rearrange("p (bp c) -> p bp c", c=128)
                km_p = km_h[hf][:].rearrange("p (bp c) -> p bp c", c=128)
                for g2 in range(2 * (g % 2), 2 * (g % 2) + 2):
                    tp = tp_ps.tile([128, 2, 128], F32, tag="tp", name=f"tp{g}_{g2}")
                    tk = tp_ps.tile([128, 2, 128], F32, tag="tp", name=f"tk{g}_{g2}")
                    for j in range(2):
                        bpl = g2 * 2 + j
                        nc.tensor.transpose(tp[:, j, :], qm_p[:, bpl, :], idq_t[:])
                        nc.tensor.transpose(tk[:, j, :], km_p[:, bpl, :], idq_t[:])
                    tpv = tp[:].rearrange("p j (ri y) -> p ri j y", ri=2)
                    tkv = tk[:].rearrange("p j (ri y) -> p ri j y", ri=2)
                    if g2 % 2 == 0:
                        nc.scalar.copy(qe_h[hf][:, :, g2 * 2:(g2 + 1) * 2, :], tpv)
                        nc.scalar.copy(ke_h[hf][:, :, g2 * 2:(g2 + 1) * 2, :], tkv)
                    else:
                        nc.vector.tensor_copy(qe_h[hf][:, :, g2 * 2:(g2 + 1) * 2, :], tpv)
                        nc.vector.tensor_copy(ke_h[hf][:, :, g2 * 2:(g2 + 1) * 2, :], tkv)
                bsl = slice(4 * (g % 2), 4 * (g % 2) + 4)
                nc.vector.tensor_scalar_mul(qf_h[hf][:, 0, bsl, :], qe_h[hf][:, 1, bsl, :], -1.0)
                nc.vector.tensor_copy(qf_h[hf][:, 1, bsl, :], qe_h[hf][:, 0, bsl, :])

            def attn1_group(g):
                hf = g // 2
                # ----- stage 4: attn1 -> X^T psum, A/B fp32 sbuf -----
                # 2 same-parity b per matmul pair (256 cols each); useful
                # quadrants j == j'; partition-aligned extraction.
                for par in range(2):
                    base = 64 * par
                    sl = slice(base, base + 64)
                    for g4 in range(2 * (g % 2), 2 * (g % 2) + 2):
                        pt = at_ps.tile([128, 2, 2, 64], F32, tag="pt", bufs=2,
                                        name=f"pt{g}_{par}_{g4}")
                        psl = slice(2 * g4, 2 * g4 + 2)
                        nc.tensor.matmul(pt[:], ke_h[hf][sl, 0, psl, :],
                                         qe_h[hf][sl, :, psl, :],
                                         start=True, stop=False)
                        nc.tensor.matmul(pt[:], ke_h[hf][sl, 1, psl, :],
                                         qf_h[hf][sl, :, psl, :],
                                         start=False, stop=True)
                        cg = 8 * hf + 2 * g4 + par
                        if (par + g4) % 2 == 0:
                            nc.scalar.copy(ab_t[0:64, cg, :].rearrange("p (ri y) -> p ri y", ri=2), pt[0:64, :, 0, :])
                            nc.vector.tensor_copy(ab_t[64:128, cg, :].rearrange("p (ri y) -> p ri y", ri=2), pt[64:128, :, 1, :])
                        else:
                            nc.vector.tensor_copy(ab_t[0:64, cg, :].rearrange("p (ri y) -> p ri y", ri=2), pt[0:64, :, 0, :])
                            nc.scalar.copy(ab_t[64:128, cg, :].rearrange("p (ri y) -> p ri y", ri=2), pt[64:128, :, 1, :])

            # Emission order = PE dependency-ready order (PE queues are
            # in-order; a stalled head blocks everything behind it).
            dft_half(0)
            for g in (0, 1):
                tr_group(g)
                attn1_group(g)
            tanh_half(0)
            dft_half(1)
            attn2_half(0, "act")
            for g in (2, 3):
                tr_group(g)
                attn1_group(g)
            tanh_half(1)
            stage789_half(0, "act", "mix")
            attn2_half(1, "mix")
            stage789_half(1, "mix", "mix")

    nc.compile()
    return nc


_NC_CACHE = None


def _get_nc():
    global _NC_CACHE
    if _NC_CACHE is None:
        _NC_CACHE = build()
    return _NC_CACHE


def _host_prep(q, k, Wr, Wi):
    """Build the 8 per-core input maps (numpy relayout/cast only)."""
    l = np.arange(L, dtype=np.float64)[:, None]
    m = np.arange(M, dtype=np.float64)[None, :]
    ang = 2.0 * np.pi * l * m / L
    F = np.concatenate([np.cos(ang), -np.sin(ang)], axis=1).astype(np.float32)  # [L, 2M]
    fh = F.astype(np.float16)
    fl = (F - fh.astype(np.float32)).astype(np.float16)
    # fh/fl as [p][(c, 2m)]
    fh = fh.reshape(NCHUNK, 128, 2 * M).transpose(1, 0, 2).reshape(128, 1024)
    fl = fl.reshape(NCHUNK, 128, 2 * M).transpose(1, 0, 2).reshape(128, 1024)

    cm = np.full(M, 2.0); cm[0] = 1.0
    ang2 = 2.0 * np.pi * m.T * np.arange(L, dtype=np.float64)[None, :] / L
    SC = 2.0 ** GSHIFT / (L * 512.0 * 512.0)
    g = np.concatenate([
        cm[:, None] * np.cos(ang2) * SC,
        -cm[:, None] * np.sin(ang2) * SC,
    ], axis=0).astype(np.float32).astype(np.float16)  # [2M, L]

    f12 = (F * 2.0 ** -12).astype(np.float16)
    f12 = f12.reshape(NCHUNK, 128, 2 * M).transpose(1, 0, 2).reshape(128, 1024)
    cpack = np.concatenate([fh, fl, g.astype(np.float16), f12], axis=1)  # [128, 4096]

    idq = np.eye(128, dtype=np.float32)

    from ml_dtypes import float8_e4m3fn as E4M3
    maps = []
    for h in range(H):
        def split(x):
            xs = np.ascontiguousarray(x[:, :, h, :].transpose(1, 0, 2)).reshape(L, B * E)
            hi = xs.astype(np.float16)
            lo = ((xs - hi.astype(np.float32)) * 2.0 ** 12).astype(E4M3)
            return hi, lo
        qh, ql8 = split(q)
        kh, kl8 = split(k)
        # pack [c][hf][p][t][col]
        xph = np.empty((NCHUNK, NHALF, 128, 2, HB * E), np.float16)
        xpl = np.empty((NCHUNK, NHALF, 128, 2, HB * E), E4M3)
        for t, (dst, src) in enumerate((( xph, qh), (xph, kh), (xpl, ql8), (xpl, kl8))):
            sv = src.reshape(NCHUNK, 128, NHALF, HB * E)
            dst[:, :, :, t % 2, :] = sv.transpose(0, 2, 1, 3)
        wpk = np.empty((E, 2, M, O), np.float32)
        wpk[:, 0] = (Wr[h] * 2.0 ** WSHIFT).transpose(0, 2, 1)  # [e,o,x]->[e,x,o]
        wpk[:, 1] = (Wi[h] * 2.0 ** WSHIFT).transpose(0, 2, 1)
        maps.append({
            "xh": xph,
            "xl": xpl,
            "cp": cpack,
            "wp": wpk.astype(np.float16),
            "idq": idq,
        })
    return maps


def kernel(q, k, v, Wr, Wi, _trace=False):
    q = np.asarray(q, np.float32)
    k = np.asarray(k, np.float32)
    Wr = np.asarray(Wr, np.float32)
    Wi = np.asarray(Wi, np.float32)
    nc = _get_nc()
    maps = _host_prep(q, k, Wr, Wi)
    try:
        res = run_bass_kernel_spmd(nc, maps, core_ids=list(range(H)), trace=_trace)
    except ModuleNotFoundError:
        res = run_bass_kernel_spmd(nc, maps, core_ids=list(range(H)), trace=False)
    # out_d[bp][p][l]: b = 2*bp + (p//64), o = p%64 -> plain b order
    out = np.empty((B, H, O, L), np.float32)
    for h in range(H):
        o = np.asarray(res.results[h]["out"], np.float32).reshape(B, O, L)
        o *= np.float32(OUT_SCALE)
        out[:, h] = o
    if _trace:
        kernel.last_results = res
    return out.astype(np.float32)
